# revision 72
# baseline (speedup 1.0000x reference)
"""GCNCritic forward kernel for Trainium2 (Bass/Tile), 8-core data-parallel.

Math collapse: the reference GCN runs on fully-connected 16-node graphs with
self-loops, so each GCN layer's output is constant across the 16 nodes of a
sample.  The two GCN layers + global_mean_pool reduce to per-sample matmuls
on the per-sample mean of x = relu(obs @ W_pre + b_pre):

    xm = mean_nodes(relu(obs @ W_pre + b_pre))            # [B, HID]
    x1 = relu(xm @ W_gcn0 + b_gcn0)                       # [B, HID]
    x2 = relu(x1 @ W_gcn1 + b_gcn1)                       # [B, HID]
    g  = relu(x2 @ W_post + b_post)                       # [B, GE]
    gz = g @ W1[:GE]                                      # [B, F1]
    loc = relu(obs @ W_loc + b_loc)                       # [B*n, LE]
    z1 = relu(loc @ W1[GE:] + gz[sample] + b1)            # [B*n, F1]
    z2 = relu(z1 @ W2 + b2)                               # [B*n, F2]
    q  = z2 @ W3            (+ b3 added host-side)        # [B*n, 8]

Sharding: batch (2048 samples) split across 8 NeuronCores, 256 samples
(4096 rows) per core; weights replicated.  All activations are kept
feature-on-partitions; every weight is consumed as lhsT in natural [K, M]
layout, so the device program contains no transposes.  obs ships
pre-transposed ([OBS, rows] per core); q is produced transposed ([8, rows])
and un-transposed host-side.

All matmul operands are bf16 (same PE rate as float32r, half the HBM
traffic); PSUM stays fp32 and biases are fp32.  Phase B runs in 4
quarter-batches so the z pipeline can start after only 2 obs tiles.  A burst
of tiny dummy matmuls at program start warms the PE p-state during the
initial DMA fill so real matmuls run at full clock.
"""

import numpy as np
import ml_dtypes

import concourse.bass as bass
import concourse.mybir as mybir
import concourse.tile as tile
from concourse.bass import ts
from concourse.bass_utils import run_bass_kernel_spmd

OBS = 128
N_AGENT = 16
HID = 128
GE = 256
LE = 256
F1 = 512
F2 = 512
NA = 8
B = 2048
NCORES = 8
BS = B // NCORES            # 256 samples per core
R = BS * N_AGENT            # 4096 rows per core
RT = 512                    # rows per tile
NT = R // RT                # 8 row tiles
SPT = RT // N_AGENT         # 32 samples per row tile
NQ = 4                      # phase-B quarters
SQ = BS // NQ               # 64 samples per quarter

F32 = mybir.dt.float32
BF = mybir.dt.bfloat16
RELU = mybir.ActivationFunctionType.Relu
ADD = mybir.AluOpType.add
MAX = mybir.AluOpType.max

# wpack (bf16) column layout
C_WPRE = 0                  # [128]
C_WLOC = 128                # [2 m][128]
C_WG0 = 384
C_WG1 = 512
C_WPOST = 640               # [2 m][128]
C_W1A = 896                 # [2 k][4 m][128]
C_W1B = 1920                # [2 k][4 m][128]
C_W2 = 2944                 # [4 k][4 m][128]
C_W3 = 4992                 # [4 k][8]
WCOLS = 5024

# bpack (f32) column layout
CB_PRE = 0
CB_G0 = 1
CB_G1 = 2
CB_POST = 3                 # 2 cols
CB_LOC = 5                  # 2 cols
CB_B1 = 7                   # 4 cols
CB_B2 = 11                  # 4 cols
BCOLS = 16

import os
N_WARM = 0 if os.environ.get("KV_NOWARM") else 120
AP_WARM = 24


def _pack_weights(i):
    wp = np.zeros((128, WCOLS), np.float32)
    wp[:, C_WPRE:C_WPRE + 128] = i["W_pre"]
    wp[:, C_WLOC:C_WLOC + 256] = i["W_loc"]
    wp[:, C_WG0:C_WG0 + 128] = i["W_gcn"][0] / N_AGENT
    wp[:, C_WG1:C_WG1 + 128] = i["W_gcn"][1]
    wp[:, C_WPOST:C_WPOST + 256] = i["W_post"]
    for k in range(2):
        for m in range(4):
            wp[:, C_W1A + (k * 4 + m) * 128:C_W1A + (k * 4 + m + 1) * 128] = \
                i["W1"][k * 128:(k + 1) * 128, m * 128:(m + 1) * 128]
            wp[:, C_W1B + (k * 4 + m) * 128:C_W1B + (k * 4 + m + 1) * 128] = \
                i["W1"][GE + k * 128:GE + (k + 1) * 128, m * 128:(m + 1) * 128]
    for k in range(4):
        for m in range(4):
            wp[:, C_W2 + (k * 4 + m) * 128:C_W2 + (k * 4 + m + 1) * 128] = \
                i["W2"][k * 128:(k + 1) * 128, m * 128:(m + 1) * 128]
        wp[:NA * 0 + 128, C_W3 + k * NA:C_W3 + (k + 1) * NA] = \
            i["W3"][k * 128:(k + 1) * 128]
    bp = np.zeros((128, BCOLS), np.float32)
    bp[:, CB_PRE] = i["b_pre"]
    bp[:, CB_G0] = i["b_gcn"][0]
    bp[:, CB_G1] = i["b_gcn"][1]
    bp[:, CB_POST:CB_POST + 2] = i["b_post"].reshape(2, 128).T
    bp[:, CB_LOC:CB_LOC + 2] = i["b_loc"].reshape(2, 128).T
    bp[:, CB_B1:CB_B1 + 4] = i["b1"].reshape(4, 128).T
    bp[:, CB_B2:CB_B2 + 4] = i["b2"].reshape(4, 128).T
    return wp.astype(ml_dtypes.bfloat16), bp


def _build():
    nc = bass.Bass("TRN2", target_bir_lowering=False, debug=False)

    obs_h = nc.dram_tensor("obs", [OBS, R], BF, kind="ExternalInput")
    wp_h = nc.dram_tensor("wpack", [128, WCOLS], BF, kind="ExternalInput")
    bp_h = nc.dram_tensor("bpack", [128, BCOLS], F32, kind="ExternalInput")
    out_h = nc.dram_tensor("out", [NA, R], F32, kind="ExternalOutput")

    with tile.TileContext(nc) as tc:
        with (
            tc.tile_pool(name="consts", bufs=1) as consts,
            tc.tile_pool(name="persist", bufs=1) as persist,
            tc.tile_pool(name="work", bufs=2) as work,
            tc.tile_pool(name="zwork", bufs=2) as zwork,
            tc.tile_pool(name="ps", bufs=8, space="PSUM") as psp,
        ):
            wp = consts.tile([128, WCOLS], BF, tag="wp")
            bp = consts.tile([128, BCOLS], F32, tag="bp")
            dumw = consts.tile([128, 16], BF, tag="dumw")
            dumr = consts.tile([128, AP_WARM], BF, tag="dumr")

            obsT = persist.tile([128, NT, RT], BF, tag="obsT")
            locT = persist.tile([128, 2, NT, RT], BF, tag="locT")
            xsum = persist.tile([128, BS], BF, tag="xsum")
            g = persist.tile([128, 2, BS], BF, tag="g")
            gz = persist.tile([128, 4, BS], BF, tag="gz")

            def wsl(c0, n=128):
                return wp[:, c0:c0 + n]

            def bia(c0):
                return bp[:, c0:c0 + 1]

            # psum pools (8 banks total): zp = z1 chunks (4 banks,
            # fine-grained rotation), cp = z2/q/B-chain/A-x (3 banks),
            # lp = loc chunks (1 bank, alternating).  Separate tags so
            # phases don't serialize on a shared rotation.
            def cp_tile(name):
                return psp.tile([128, 512], F32, tag="cp", bufs=3, name=name)

            def lp_tile(name):
                return psp.tile([128, 512], F32, tag="lp", bufs=1, name=name)

            def xp_tile(name):
                return psp.tile([128, 512], F32, tag="xp", bufs=1, name=name)

            def zp_tile(name):
                return psp.tile([128, 512], F32, tag="zp", bufs=3, name=name)

            # ---- PE warm-up: tiny matmuls while the first DMAs stream ----
            nc.vector.memset(dumw, 0.0)
            nc.vector.memset(dumr, 0.0)
            warmp = cp_tile("warm")
            for _ in range(N_WARM):
                nc.tensor.matmul(warmp[:16, :AP_WARM], dumw, dumr,
                                 start=True, stop=True)

            # ---- DMAs, in SP program order (= transfer order): obs0 and
            # the weight blocks on tile-0's critical chain first, then the
            # obs stream self-paces phase A ----
            nc.sync.dma_start(obsT[:, 0, :], obs_h[:, ts(0, RT)])
            nc.sync.dma_start(wp[:, :C_WLOC + 256], wp_h[:, :C_WLOC + 256])
            nc.sync.dma_start(bp, bp_h[:, :])
            nc.sync.dma_start(wp[:, C_W1B:C_W2], wp_h[:, C_W1B:C_W2])
            nc.sync.dma_start(wp[:, C_WG0:C_W1A], wp_h[:, C_WG0:C_W1A])
            nc.sync.dma_start(wp[:, C_W1A:C_W1B], wp_h[:, C_W1A:C_W1B])
            nc.sync.dma_start(obsT[:, 1, :], obs_h[:, ts(1, RT)])
            nc.sync.dma_start(wp[:, C_W2:C_W2 + 1024], wp_h[:, C_W2:C_W2 + 1024])
            nc.sync.dma_start(obsT[:, 2, :], obs_h[:, ts(2, RT)])
            nc.sync.dma_start(wp[:, C_W2 + 1024:], wp_h[:, C_W2 + 1024:])
            for t in range(3, NT):
                nc.sync.dma_start(obsT[:, t, :], obs_h[:, ts(t, RT)])

            # ---- phase A: one row tile -> x_sb, locT, xsum ----
            def phase_A(t):
                xp = xp_tile("axp")
                nc.tensor.matmul(xp, wsl(C_WPRE), obsT[:, t, :],
                                 start=True, stop=True)
                lp0 = lp_tile("lp0")
                nc.tensor.matmul(lp0, wsl(C_WLOC), obsT[:, t, :],
                                 start=True, stop=True)
                x_sb = work.tile([128, RT], BF, tag="xsb", name="xsb")
                nc.scalar.activation(x_sb, xp, RELU, bias=bia(CB_PRE))
                nc.scalar.activation(locT[:, 0, t, :], lp0, RELU,
                                     bias=bia(CB_LOC))
                lp1 = lp_tile("lp1")
                nc.tensor.matmul(lp1, wsl(C_WLOC + 128), obsT[:, t, :],
                                 start=True, stop=True)
                nc.vector.tensor_scalar(locT[:, 1, t, :], lp1,
                                        bia(CB_LOC + 1), 0.0,
                                        op0=ADD, op1=MAX)
                xv = x_sb.rearrange("p (s k) -> p s k", k=N_AGENT)
                if t < 2:
                    # tile 0/1 gate the z-pipeline start: lowest-latency path
                    with nc.allow_low_precision(reason="16-elem sample sums"):
                        nc.vector.tensor_reduce(
                            xsum[:, ts(t, SPT)], xv,
                            axis=mybir.AxisListType.X, op=ADD)
                else:
                    # gpsimd halves the agent dim once (it is otherwise
                    # idle), DVE finishes the 8-way reduce at half the size
                    r8 = work.tile([128, SPT, 8], BF, tag="r8", name="r8")
                    nc.gpsimd.tensor_tensor(r8, xv[:, :, :8], xv[:, :, 8:],
                                            op=ADD)
                    with nc.allow_low_precision(reason="16-elem sample sums"):
                        nc.vector.tensor_reduce(
                            xsum[:, ts(t, SPT)], r8,
                            axis=mybir.AxisListType.X, op=ADD)

            # ---- phase B: per-sample chain on samples [s0, s0+n) ----
            def phase_B(s0, n, fast=False):
                # fast=True routes the chain evacs through the idle ACT
                # engine (lowest latency); the in-spine chunks use DVE
                def ev(dst, src, c0):
                    if fast:
                        nc.scalar.activation(dst, src, RELU, bias=bia(c0))
                    else:
                        nc.vector.tensor_scalar(dst, src, bia(c0), 0.0,
                                                op0=ADD, op1=MAX)
                S = slice(s0, s0 + n)
                x1p = xp_tile("x1p")
                nc.tensor.matmul(x1p[:, :n], wsl(C_WG0), xsum[:, S],
                                 start=True, stop=True)
                x1t = work.tile([128, SQ], BF, tag="x1t", name="x1t")
                ev(x1t[:, :n], x1p[:, :n], CB_G0)
                x2p = xp_tile("x2p")
                nc.tensor.matmul(x2p[:, :n], wsl(C_WG1), x1t[:, :n],
                                 start=True, stop=True)
                x2t = work.tile([128, SQ], BF, tag="x2t", name="x2t")
                ev(x2t[:, :n], x2p[:, :n], CB_G1)
                gp = xp_tile("gp")
                for m in range(2):
                    nc.tensor.matmul(gp[:, ts(m, n)], wsl(C_WPOST + m * 128),
                                     x2t[:, :n], start=True, stop=True)
                for m in range(2):
                    ev(g[:, m, S], gp[:, ts(m, n)], CB_POST + m)
                gzp = xp_tile("gzp")
                for m in range(4):
                    for k in range(2):
                        nc.tensor.matmul(
                            gzp[:, ts(m, n)],
                            wsl(C_W1A + (k * 4 + m) * 128), g[:, k, S],
                            start=(k == 0), stop=(k == 1))
                nc.vector.tensor_copy(
                    gz[:, :, S],
                    gzp[:, :4 * n].rearrange("p (m s) -> p m s", m=4))

            # ---- spine: z1 chunk m for tile t (mm -> +gz+b1 -> relu) ----
            def z1_chunk(t, z1, m):
                zp = zp_tile("zp")
                for k in range(2):
                    nc.tensor.matmul(
                        zp, wsl(C_W1B + (k * 4 + m) * 128),
                        locT[:, k, t, :],
                        start=(k == 0), stop=(k == 1))
                nc.vector.scalar_tensor_tensor(
                    z1[:, m, :].rearrange("p (s k) -> p s k", k=N_AGENT),
                    zp.rearrange("p (s k) -> p s k", k=N_AGENT),
                    bia(CB_B1 + m),
                    gz[:, m, ts(t, SPT)][:, :, None].to_broadcast(
                        [128, SPT, N_AGENT]),
                    op0=ADD, op1=ADD)
                nc.gpsimd.tensor_scalar_max(z1[:, m, :], z1[:, m, :], 0.0)

            def z1_tile(t):
                z1 = zwork.tile([128, 4, RT], BF, tag="z1", name="z1")
                for m in range(4):
                    z1_chunk(t, z1, m)
                return z1

            def z2_q(t, z1, z1next=None, tnext=None, c0=0, c1=RT):
                # z2+q for rows [c0, c1) of tile t, with tile (t+2)'s z1
                # chunks interleaved between the z2 m-groups: every engine
                # queue then holds independent work between a psum fill and
                # its reuse.
                n = c1 - c0
                z2 = zwork.tile([128, 4, RT], BF, tag="z2", name="z2")
                for m in range(4):
                    zp = cp_tile("z2p")
                    for k in range(4):
                        nc.tensor.matmul(
                            zp[:, :n], wsl(C_W2 + (k * 4 + m) * 128),
                            z1[:, k, c0:c1],
                            start=(k == 0), stop=(k == 3))
                    if t == NT - 1 and m % 2 == 1:
                        # final short sub-tile: alternate evac engines so the
                        # serial tail pipeline is two-wide
                        nc.vector.tensor_scalar(z2[:, m, c0:c1], zp[:, :n],
                                                bia(CB_B2 + m), 0.0,
                                                op0=ADD, op1=MAX)
                    else:
                        nc.scalar.activation(z2[:, m, c0:c1], zp[:, :n], RELU,
                                             bias=bia(CB_B2 + m))
                    if z1next is not None:
                        z1_chunk(tnext, z1next, m)
                # q runs after all four z2 evacs, at the tile tail, so qp's
                # bank hold never blocks the z2 group pipeline
                qp = cp_tile("qp")
                for m in range(4):
                    nc.tensor.matmul(qp[:NA, :n], wp[:, C_W3 + m * NA:
                                                     C_W3 + (m + 1) * NA],
                                     z2[:, m, c0:c1], start=(m == 0),
                                     stop=(m == 3))
                qsb = zwork.tile([NA, RT], F32, tag="qsb", name="qsb")
                nc.scalar.copy(qsb[:, c0:c1], qp[:NA, :n])
                nc.sync.dma_start(out_h[:, t * RT + c0:t * RT + c1],
                                  qsb[:, c0:c1])

            # ---- schedule: phase B runs as per-tile 32-sample chunks,
            # each right after its A tile, spreading the serial chain thin ----
            z1t = {}
            phase_A(0)
            phase_B(0, SPT, fast=True)
            z1t[0] = z1_tile(0)
            phase_A(1)
            phase_B(SPT, SPT)
            z1t[1] = z1_tile(1)
            phase_A(2)
            phase_B(2 * SPT, SPT)
            phase_A(3)
            phase_B(3 * SPT, SPT)
            for t in range(NT):
                z1n = None
                if t + 2 < NT:
                    z1n = zwork.tile([128, 4, RT], BF, tag="z1", name="z1")
                    z1t[t + 2] = z1n
                if t == NT - 1:
                    # last tile: drain a short 128-row sub-tile last so the
                    # serial evac->q->copy->DMA tail is short
                    z2_q(t, z1t[t], c0=0, c1=RT - 128)
                    z2_q(t, z1t.pop(t), c0=RT - 128, c1=RT)
                else:
                    z2_q(t, z1t.pop(t), z1n, t + 2)
                if t < 4:
                    phase_A(4 + t)
                    phase_B((4 + t) * SPT, SPT)

    _split_waits(nc)
    return nc


def _split_waits(nc):
    # walrus accepts only one sync-wait per instruction in this build; move
    # extra waits onto same-engine sequencer nops placed immediately before
    # the instruction (program order on the engine's queue, so semantics are
    # identical).
    for blk in nc.m.functions[0].blocks:
        new = []
        for inst in blk.instructions:
            if inst.sync_info is not None:
                w = list(inst.sync_info.on_wait)
                if len(w) > 1:
                    for wx in w[:-1]:
                        new.append(
                            mybir.InstNoOp(
                                name=nc.get_next_instruction_name(),
                                engine=inst.engine,
                                sync_info=mybir.SyncInfo(
                                    on_wait=[wx], on_update=[]),
                                bass_nofuse=True))
                    inst.sync_info.on_wait = [w[-1]]
            new.append(inst)
        blk.instructions[:] = new


_CACHE = {}


def _get_nc():
    if "nc" not in _CACHE:
        _CACHE["nc"] = _build()
    return _CACHE["nc"]


def kernel(trace=False, **inputs):
    obs_j = np.ascontiguousarray(np.asarray(inputs["obs_j"], dtype=np.float32))
    np_in = {
        k: np.asarray(v, dtype=np.float32)
        for k, v in inputs.items()
        if k != "obs_j"
    }
    wpack, bpack = _pack_weights(np_in)
    wpack = np.ascontiguousarray(wpack)
    bpack = np.ascontiguousarray(bpack)
    nc = _get_nc()
    in_maps = []
    for c in range(NCORES):
        ob = obs_j[c * BS:(c + 1) * BS].reshape(R, OBS).T
        in_maps.append({
            "obs": np.ascontiguousarray(ob.astype(ml_dtypes.bfloat16)),
            "wpack": wpack,
            "bpack": bpack,
        })
    res = run_bass_kernel_spmd(
        nc, in_maps, core_ids=list(range(NCORES)), trace=trace
    )
    out = np.concatenate([r["out"] for r in res.results], axis=1)  # [NA, B*n]
    q = np.ascontiguousarray(out.T).reshape(B, N_AGENT, NA)
    q = q + np_in["b3"]
    if trace:
        return q, res
    return q
